# revision 64
# baseline (speedup 1.0000x reference)
"""Additive-attention pooling kernel for 8 TRN2 NeuronCores.

Per full input:
    u = tanh(value @ W1^T + query @ W2^T + b)          # [B, S, H]
    scores = u @ w, masked to s < lens[b], softmax over s
    out = sum_s softmax(scores)[b, s] * value[b, s, :]  # [B, DV]

Design (vs the 90.9us session baseline):

1. Length-adaptive work elision: whole 512-position units past lens[b]
   are neither loaded nor computed.  kernel() deals batches onto cores
   by active-unit count (rank-dealt slot-max pattern, provably minimal
   for whole-batch slots); one program per pattern, cached.

2. Value loads as SWDGE cast-DMAs (f32->bf16) with position layout
   s = 4p + q inside each unit, making every partition's source rows
   4KB-contiguous (4x fewer descriptors -- gpsimd descriptor generation
   was the DMA serializer).  The first chunk is split into quarter-unit
   DMAs to cut the PE head-wait.  Identities and the mask iota are
   host-packed into params so the gpsimd queue never swaps ucode
   libraries mid-stream.

3. fp8 DoubleRow u-matmuls: W1 is packed host-side as the DR lhsT
   [p, i(=k-parity), hh, m]; the f32-bitcast pair transposes feed a
   strided DVE cast-copy that de-interleaves v = 2c + i pairs into the
   plane-separated [K, 2, N] rhs DoubleRow requires.  Halves the
   u-matmul stream (rel err 1.81e-2, within the 2e-2 gate; flip
   fp8=False in build_nc for the bf16 fallback at ~3.4e-3).

4. Single-bank score accumulation (both h-halves accumulate into one
   PSUM bank per 32-row sub band -> one exp instead of exp*exp),
   partition-sliced e-transposes, inline per-half-batch pooling into
   col-group bands of one PSUM bank, and per-slot unnormalized
   (pool, denominator) outputs combined on the host (exact, since
   scores use plain exp without max subtraction).
   PSUM: tp 2 banks, up 2x[128,1024], se 1, pool 1.
"""

import numpy as np
import ml_dtypes

import concourse.bass as bass
import concourse.bacc as bacc
import concourse.tile as tile
from concourse import mybir
from concourse.bass_utils import run_bass_kernel_spmd


B, S, DV, DQ, H = 32, 4096, 256, 256, 256
NCORES = 8
BL = B // NCORES  # batches per core

ST = 32           # s-tiles of 128 per batch
TPC = 8           # s-tiles per pipeline chunk
NCHUNK = BL * (ST // TPC)  # 16 chunks per core
PW = 2508         # params: w1t 512 | w2t 512 | w 2 | b 2 | qT 8 | w1 fp8
                  # bytes 128 | ident_f32 128 | ident_bf 64 | iota_s 4x32
                  # | slot-0 units 0-1 value bf16 1024 (static DMA beats
                  # the SWDGE first-chunk latency by ~5us)
F32 = mybir.dt.float32
BF16 = mybir.dt.bfloat16
F8 = mybir.dt.float8e4
I32 = mybir.dt.int32


def build_nc(pattern=(4, 4, 4, 4), fp8=True):
    nc = bacc.Bacc("TRN2", target_bir_lowering=False)

    value_ext = nc.declare_dram_parameter("value", [BL, S, DV], F32, isOutput=False)
    lens_ext = nc.declare_dram_parameter("lens", [BL], I32, isOutput=False)
    params_ext = nc.declare_dram_parameter("params", [128, PW], F32, isOutput=False)
    out_ext = nc.declare_dram_parameter("out", [BL, DV + 1], F32,
                                        isOutput=True)

    Tanh = mybir.ActivationFunctionType.Tanh
    Exp = mybir.ActivationFunctionType.Exp
    Alu = mybir.AluOpType
    DR = mybir.MatmulPerfMode.DoubleRow

    with tile.TileContext(nc) as tc:
        with (
            tc.tile_pool(name="singles", bufs=1) as singles,
            tc.tile_pool(name="nat", bufs=BL) as nat_pool,
            tc.tile_pool(name="vt", bufs=5) as vt_pool,
            tc.tile_pool(name="ut", bufs=5) as ut_pool,
            tc.tile_pool(name="esb", bufs=3) as esb_pool,
            tc.tile_pool(name="erep", bufs=4) as erep_pool,
        ):
            # ---- first two value DMAs ahead of the iotas so the DMA
            # engines start filling chunk 0 during the iota/identity prep ----
            nat = []
            for b in range(BL):
                natb = nat_pool.tile([128, ST, DV], BF16, tag="nat",
                                     name=f"nat{b}")
                nat.append(natb)
            # position layout within each 512-unit: s = 4p + q, so each
            # partition's source rows are 4KB-contiguous (4x fewer SWDGE
            # descriptors than the s = 128t + p layout).  The first chunk
            # is split into quarter-unit pieces so its first transposes
            # start ~3us earlier (the PE head-wait is one piece, not a
            # whole 256KB unit).
            # units 0-1 of slot 0 ride the params static DMA (chunk 0
            # never waits on SWDGE)

            # ---- remaining value loads: SWDGE cast-DMAs, 256KB each,
            # in round-robin (consumption) order across slots ----
            maxch = max(pattern)
            for ch in range(maxch):
                for b in range(BL):
                    if ch >= pattern[b] or (b == 0 and ch < 2):
                        continue
                    src = value_ext[b, ch * 512:(ch + 1) * 512, :]
                    nc.gpsimd.dma_start(
                        out=nat[b][:, ch * 4:(ch + 1) * 4, :],
                        in_=src.rearrange("(p q) v -> p q v", q=4),
                    )

            # split the params load into two SEPARATE tiles so readers of
            # the transpose-critical slice (w1 fp8 | identities | iota |
            # embedded value units, cols 1036:2508, sync queue) never wait
            # on the other slice's DMA (w1t/w2t/w/b/qT, scalar queue)
            params_a = singles.tile([128, PW - 1036], F32, tag="params_a")
            nc.sync.dma_start(out=params_a, in_=params_ext[:, 1036:PW])
            params_b = singles.tile([128, 1036], F32, tag="params_b")
            nc.scalar.dma_start(out=params_b, in_=params_ext[:, 0:1036])

            w1t_f = params_b[:, 0:512].rearrange("p (c h) -> p c h", c=2)
            w2t_f = params_b[:, 512:1024].rearrange("p (c h) -> p c h", c=2)
            w_f = params_b[:, 1024:1026]
            b_sb = params_b[:, 1026:1028]
            qT = params_b[:, 1028:1036].rearrange("p (c b) -> p c b", c=2)
            w1q = params_a[:, 0:128].bitcast(F8).rearrange(
                "p (i hh m) -> p i hh m", i=2, hh=2
            )
            # identities + mask iota are host-packed: keeps the gpsimd
            # queue free for SWDGE descriptor generation (no iota library
            # swap mid-stream)
            ident_f32 = params_a[:, 128:256]
            ident_bf = params_a[:, 256:320].bitcast(BF16)
            # per-slot absolute-position iota
            iota_s = params_a[:, 320:448].rearrange(
                "p (sl t) -> p sl t", sl=BL
            )
            nat00 = params_a[:, 448:1472].bitcast(BF16).rearrange(
                "p (t v) -> p t v", t=8
            )

            def nat_tile(b, t):
                if b == 0 and t < 8:
                    return nat00[:, t, :]
                return nat[b][:, t, :]

            lens_i = singles.tile([128, BL], I32, tag="lens_i")
            nc.sync.dma_start(
                out=lens_i,
                in_=bass.AP(tensor=lens_ext, offset=0, ap=[[0, 128], [1, BL]]),
            )
            lens_f = singles.tile([128, BL], F32, tag="lens_f")
            nc.vector.tensor_copy(lens_f, lens_i)

            w1t_bf = singles.tile([128, 2, H], BF16, tag="w1t_bf")
            nc.vector.tensor_copy(w1t_bf, w1t_f)

            # w replicated to 32 columns for the M=32 scores matmuls
            zero32 = singles.tile([128, 32], BF16, tag="zero32")
            nc.vector.memset(zero32, 0.0)
            w_rep = singles.tile([128, 2, 32], BF16, tag="w_rep")
            for hh in range(2):
                nc.vector.tensor_scalar(
                    w_rep[:, hh, :], zero32, w_f[:, hh:hh + 1], None, Alu.add
                )

            # 1/32-filled stationary: the mask op's accum_out counts each e
            # 32x (broadcast reps), the denominator matmul divides it back
            ones_rep = singles.tile([128, 32], BF16, tag="ones_rep")
            nc.vector.memset(ones_rep, 1.0 / 32.0)

            # c[b, h] = query[b] @ W2^T + b   ->  cT [128h, hh, b] f32
            cT = singles.tile([128, 2, BL], F32, tag="cT")
            with tc.tile_pool(name="ct_ps", bufs=2, space="PSUM") as ct_pool:
                for hh in range(2):
                    ct_ps = ct_pool.tile([128, BL], F32, tag="ct")
                    for c in range(2):
                        nc.tensor.matmul(
                            ct_ps,
                            w2t_f[:, c, hh * 128:(hh + 1) * 128],
                            qT[:, c, :],
                            start=(c == 0),
                            stop=(c == 1),
                        )
                    nc.vector.tensor_scalar(
                        cT[:, hh, :], ct_ps, b_sb[:, hh:hh + 1], None, Alu.add
                    )

            psums = singles.tile([128, BL, 8], F32, tag="psums")
            nc.vector.memset(psums, 0.0)
            psum_r = singles.tile([128, BL], F32, tag="psum_r")
            psum_bf = singles.tile([128, BL], BF16, tag="psum_bf")
            out_sb = singles.tile([128, DV + 1], F32, tag="out_sb")

            # pattern[b] counts 4-tile (512-position) units; chunks pair
            # them, with a trailing half-chunk for odd unit counts
            chunk_list = []
            nchs = [(pattern[b] + 1) // 2 for b in range(BL)]
            for g in range(max(nchs) if nchs else 0):
                for b in range(BL):
                    if g >= nchs[b]:
                        continue
                    nsub = 2 if 2 * g + 2 <= pattern[b] else 1
                    chunk_list.append((b, g, nsub))
            hbs = []
            for b in range(BL):
                ks = [i for i, c in enumerate(chunk_list) if c[0] == b]
                for hbl in range(0, len(ks), 2):
                    grp = ks[hbl:hbl + 2]
                    subs = [(k, sc) for k in grp
                            for sc in range(chunk_list[k][2])]
                    hbs.append((b, hbl // 2, grp, subs))
            s_at, exp_at, et_at = {}, {}, {}
            for hb, (b, hbl, grp, subs) in enumerate(hbs):
                close = max(grp)
                s_at[close + 1] = hb
                exp_at[close + 2] = hb
                et_at[close + 3] = hb

            uts = [None] * len(chunk_list)
            vts = {}
            se_tiles = {}
            esb_tiles = {}
            erep = [None] * BL
            tmax = [4 * pattern[b] for b in range(BL)]

            with (
                tc.tile_pool(name="tp_ps", bufs=2, space="PSUM") as tp_pool,
                tc.tile_pool(name="up_ps", bufs=2, space="PSUM") as up_pool,
                tc.tile_pool(name="se_ps", bufs=1, space="PSUM") as se_pool,
                tc.tile_pool(name="po_ps", bufs=1, space="PSUM") as po_pool,
            ):
                po_ps = po_pool.tile([128, 512], F32, tag="po")

                def emit_T(k):
                    b, g, nsub = chunk_list[k]
                    if fp8:
                        # f32-bitcast pair transposes (one 2-pass LDW moves
                        # both v-parities); the PSUM->SBUF cast copy
                        # de-interleaves pairs into the plane-separated
                        # vt[c, sub, i, pos] (v = 2c + i) [K, 2, N] rhs
                        # form DoubleRow requires
                        vt = vt_pool.tile([128, TPC // 4, 2, 512], F8,
                                          tag="vt", name=f"vt{k}")
                        vts[k] = vt
                        for h in range(nsub):
                            tp = tp_pool.tile([128, 4, 128], F32, tag="tp",
                                              name=f"tp{k}_{h}")
                            for tl in range(4):
                                t = TPC * g + 4 * h + tl
                                nc.tensor.matmul(
                                    tp[:, tl, :],
                                    nat_tile(b, t).bitcast(F32),
                                    ident_f32,
                                    is_transpose=True,
                                    start=(tl == 0),
                                    stop=(tl == 3),
                                )
                            tpb = tp.bitcast(BF16)
                            src = bass.AP(
                                tensor=tpb.tensor, offset=tpb.offset,
                                ap=[tpb.ap[0], [1, 2], [256, 4], [2, 128]],
                            )
                            nc.vector.tensor_copy(vt[:, h, :, :], src)
                        return
                    vt = vt_pool.tile([128, TPC, 256], BF16, tag="vt",
                                      name=f"vt{k}")
                    vts[k] = vt
                    for h in range(nsub):
                        tp = tp_pool.tile([128, 4, 128], F32, tag="tp",
                                          name=f"tp{k}_{h}")
                        for tl in range(4):
                            t = TPC * g + 4 * h + tl
                            nc.tensor.matmul(
                                tp[:, tl, :],
                                nat_tile(b, t).bitcast(F32),
                                ident_f32,
                                is_transpose=True,
                                start=(tl == 0),
                                stop=(tl == 3),
                            )
                        nc.vector.tensor_copy(
                            vt[:, 4 * h:4 * h + 4, :], tp.bitcast(BF16)
                        )

                def emit_U(k):
                    b, g, nsub = chunk_list[k]
                    vt_base = vts[k][:, :, :]
                    ut = ut_pool.tile([128, 2, 1024], BF16, tag="ut",
                                      name=f"ut{k}")
                    uts[k] = ut
                    for hh in range(2):
                        up = up_pool.tile([128, 1024], F32, tag="up",
                                          name=f"up{k}_{hh}")
                        for sc in range(nsub):
                            if fp8:
                                nc.tensor.matmul(
                                    up[:, sc * 512:(sc + 1) * 512],
                                    w1q[:, :, hh, :],
                                    vts[k][:, sc, :, :],
                                    start=True,
                                    stop=True,
                                    perf_mode=DR,
                                )
                            else:
                                for i in range(2):
                                    rhs = bass.AP(
                                        tensor=vt_base.tensor,
                                        offset=vt_base.offset + sc * 1024 + i,
                                        ap=[vt_base.ap[0], [2, 512]],
                                    )
                                    nc.tensor.matmul(
                                        up[:, sc * 512:(sc + 1) * 512],
                                        w1t_bf[:, i, hh * 128:(hh + 1) * 128],
                                        rhs,
                                        start=(i == 0),
                                        stop=(i == 1),
                                    )
                        nc.scalar.activation(
                            ut[:, hh, 0:512 * nsub], up[:, 0:512 * nsub],
                            Tanh, bias=cT[:, hh, b:b + 1], scale=1.0,
                        )

                def emit_S(hb):
                    # both h-halves accumulate into one PSUM bank per band;
                    # hh-outer order puts the 4 distinct col-groups
                    # back-to-back so their streams overlap in the array
                    b, hbl, grp, subs = hbs[hb]
                    se_t = se_pool.tile([128, 512], F32, tag="se",
                                        name=f"se{hb}")
                    se_tiles[hb] = se_t
                    for hh in range(2):
                        for r, (j, sc) in enumerate(subs):
                            nc.tensor.matmul(
                                se_t[32 * r:32 * r + 32, :],
                                w_rep[:, hh, :],
                                uts[j][:, hh, sc * 512:(sc + 1) * 512],
                                start=(hh == 0),
                                stop=(hh == 1),
                                tile_position=(0, 32 * r),
                            )

                def emit_EXP(hb):
                    L = len(hbs[hb][3])
                    se_t = se_tiles[hb]
                    e_sb = esb_pool.tile([128, 512], BF16, tag="esb",
                                         name=f"esb{hb}")
                    esb_tiles[hb] = e_sb
                    nc.scalar.activation(
                        e_sb[0:32 * L, :], se_t[0:32 * L, :], Exp
                    )

                def emit_ET(hb):
                    b, hbl, grp, subs = hbs[hb]
                    L = len(subs)
                    e_sb = esb_tiles[hb]
                    et = tp_pool.tile([128, 1024], BF16, tag="tp",
                                      name=f"et{hb}")
                    for tl in range(4):
                        nc.tensor.matmul(
                            et[:, tl * 128:tl * 128 + 32 * L],
                            e_sb[0:32 * L, 128 * tl:128 * (tl + 1)],
                            ident_bf[0:32 * L, 0:32 * L],
                            is_transpose=True,
                            start=(tl == 0),
                            stop=(tl == 3),
                        )
                    # e_rep[b][:, t, :] = mask * e broadcast to 32 reps, for
                    # t = 16*(hb%2) + 8*r1 + 4*r0 + tl at et[:, 256*tl + 32*(2*r1+r0)]
                    if hbl == 0:
                        erep[b] = erep_pool.tile([128, ST, 32], BF16,
                                                 tag="erep", name=f"erep{b}")
                    et_all = et[:, :]
                    for r in range(len(subs)):
                        tbase = 16 * hbl + 4 * r
                        in1 = bass.AP(
                            tensor=et_all.tensor, offset=et_all.offset + 32 * r,
                            ap=[et_all.ap[0], [128, 4], [0, 32]],
                        )
                        iosl = iota_s[:, b, tbase:tbase + 4]
                        io_b = bass.AP(
                            tensor=iosl.tensor, offset=iosl.offset,
                            ap=[iosl.ap[0], [1, 4], [0, 32]],
                        )
                        osl = erep[b][:, tbase:tbase + 4, :]
                        slot = hbl * 4 + r
                        nc.vector.scalar_tensor_tensor(
                            osl,
                            io_b,
                            lens_f[:, b:b + 1],
                            in1,
                            Alu.is_lt,
                            Alu.mult,
                            accum_out=psums[:, b, slot:slot + 1],
                        )

                pool_pending = {b: [] for b in range(BL)}

                def emit_pool_mm(b, t):
                    nc.tensor.matmul(
                        po_ps[32 * b:32 * b + 32, 0:DV],
                        erep[b][:, t, :],
                        nat_tile(b, t),
                        start=(t == 0),
                        stop=(t == tmax[b] - 1),
                        tile_position=(0, 32 * b),
                    )
                    if t == tmax[b] - 1:
                        emit_OUT(b)

                def pool_step(force=False):
                    # drain pending pool tiles round-robin across batches:
                    # adjacent matmuls then target different col-groups and
                    # their N=256 streams overlap in the array (~2x).
                    # Tiles of a lone batch are held until a second batch
                    # has pending work (or the final forced drain).
                    while True:
                        active = [b for b in range(BL) if pool_pending[b]]
                        if not active or (len(active) < 2 and not force):
                            break
                        for b in active:
                            emit_pool_mm(b, pool_pending[b].pop(0))

                def emit_OUT(b):
                    # per-slot unnormalized (pool, denominator) copy-out
                    # as soon as this slot's pool accumulation closes;
                    # the host divides after summing across cores/slots
                    rows = slice(32 * b, 32 * b + 32)
                    nc.vector.tensor_reduce(
                        psum_r[:, b:b + 1], psums[:, b, :], op=Alu.add,
                        axis=mybir.AxisListType.X,
                    )
                    nc.vector.tensor_copy(
                        psum_bf[:, b:b + 1], psum_r[:, b:b + 1]
                    )
                    nc.tensor.matmul(
                        po_ps[rows, DV:DV + 1],
                        ones_rep,
                        psum_bf[:, b:b + 1],
                        start=True,
                        stop=True,
                        tile_position=(0, 32 * b),
                    )
                    nc.vector.tensor_copy(
                        out_sb[rows], po_ps[rows, 0:DV + 1]
                    )

                pool_at = {}
                for hb, (b, hbl, grp, subs) in enumerate(hbs):
                    pool_at[max(grp) + 4] = hb

                NK = len(chunk_list)
                for k in range(NK + 5):
                    if k in exp_at:
                        emit_EXP(exp_at[k])
                    if k < NK:
                        emit_T(k)
                    if k in et_at:
                        emit_ET(et_at[k])
                    if k < NK:
                        emit_U(k)
                    if k in s_at:
                        emit_S(s_at[k])
                    if k in pool_at:
                        hb = pool_at[k]
                        b, hbl, grp, subs = hbs[hb]
                        pool_pending[b].extend(
                            range(16 * hbl, 16 * hbl + 4 * len(subs))
                        )
                    pool_step()
                pool_step(force=True)
                for b in range(BL):
                    if pattern[b] == 0:
                        # phantom slot: emit zeros so the host-side sum
                        # is unaffected
                        nc.vector.memset(
                            out_sb[32 * b:32 * b + 32, :], 0.0
                        )

                ob_rows = out_sb.rearrange("(a b) s -> a b s", b=32)[:, 0, :]
                nc.sync.dma_start(out=out_ext[:, :], in_=ob_rows)

    nc.compile()
    return nc


_NC_CACHE = {}


def _get_nc(pattern):
    if pattern not in _NC_CACHE:
        _NC_CACHE[pattern] = build_nc(pattern)
    return _NC_CACHE[pattern]


def plan_assignment(lens):
    """Deal batches (sorted by active chunks, desc) round-robin into cores.

    Returns (assign[core][slot] = batch index, pattern) where pattern[j] is
    the max chunk count over cores at slot j -- the shared compiled shape
    (provably minimal for whole-batch slots).
    """
    g = np.clip(np.ceil(np.asarray(lens) / 512).astype(int), 1, 8)
    order = np.argsort(-g, kind="stable")
    assign = [[None] * BL for _ in range(NCORES)]
    for rank, b in enumerate(order):
        assign[rank % NCORES][rank // NCORES] = int(b)
    pattern = tuple(
        int(max(g[assign[i][j]] for i in range(NCORES))) for j in range(BL)
    )
    return assign, pattern


def make_in_maps(value, query, lens, W1, W2, b, w, assign):
    value = np.ascontiguousarray(np.asarray(value, dtype=np.float32))
    query = np.asarray(query, dtype=np.float32)
    lens = np.ascontiguousarray(np.asarray(lens, dtype=np.int32))
    w1t = np.asarray(W1, dtype=np.float32).T
    f8 = ml_dtypes.float8_e4m3
    w1_q = np.asarray(W1, dtype=np.float32).astype(f8)   # [h, v]
    # [p, i, hh, m] = W1_q[128*hh + m, 2p + i]
    wq = w1_q.reshape(2, 128, 128, 2)                     # [hh, m, p, i]
    wq = np.ascontiguousarray(wq.transpose(2, 3, 0, 1))   # [p, i, hh, m]
    w1_bytes = wq.reshape(128, 512).view(np.float32)      # [128, 128]
    w2t = np.asarray(W2, dtype=np.float32).T
    bvec = np.asarray(b, dtype=np.float32).reshape(H)
    wvec = np.asarray(w, dtype=np.float32).reshape(H)

    def pack(core):
        sel = assign[core]
        P = np.zeros((128, PW), np.float32)
        # w1t by v-parity: col (i*256 + hh*128 + m) = W1T[2p+i, 128hh+m]
        w1p = w1t.reshape(128, 2, 2, 128)          # [p2, i, hh, m] with v=2*p2+i
        P[:, 0:512] = w1p.transpose(0, 1, 2, 3).reshape(128, 512)
        P[:, 512:1024] = w2t.reshape(2, 128, H).transpose(1, 0, 2).reshape(128, 512)
        P[:, 1024:1026] = wvec.reshape(2, 128).T
        P[:, 1026:1028] = bvec.reshape(2, 128).T
        P[:, 1028:1036] = (
            query[sel].T.reshape(2, 128, BL).transpose(1, 0, 2)
            .reshape(128, 2 * BL)
        )
        P[:, 1036:1164] = w1_bytes
        P[:, 1164:1292] = np.eye(128, dtype=np.float32)
        P[:, 1292:1356] = (
            np.eye(128, dtype=ml_dtypes.bfloat16).reshape(128, 64, 2)
            .view(np.float32).reshape(128, 64)
        )
        # mask iota per slot: tile t = (u, q) holds position 512u + 4p + q
        p_i = np.arange(128)[:, None]
        u_i = (np.arange(32) // 4)[None, :]
        q_i = (np.arange(32) % 4)[None, :]
        io = (512 * u_i + 4 * p_i + q_i).astype(np.float32)
        P[:, 1356:1484] = np.tile(io, (1, BL))
        # slot-0 units 0-1 value as bf16 in the s = 4p + q layout
        v00 = value[sel[0], 0:1024, :].astype(ml_dtypes.bfloat16)
        P[:, 1484:2508] = (
            v00.reshape(2, 128, 4, 256).transpose(1, 0, 2, 3)
            .reshape(128, 2048).view(np.float32)
        )
        return np.ascontiguousarray(P)

    in_maps = []
    for i in range(NCORES):
        sel = assign[i]
        in_maps.append({
            "value": np.ascontiguousarray(value[sel]),
            "lens": np.ascontiguousarray(lens[sel]),
            "params": pack(i),
        })
    return in_maps


def _axon_reset():
    # clear a wedged exec unit left over from a previous crashed run
    try:
        import ctypes
        import jax
        jax.devices()
        lib = ctypes.CDLL("/opt/axon/libaxon_pjrt.so")
        lib.axon_reset.restype = ctypes.c_int64
        lib.axon_reset()
    except Exception:
        pass


def kernel(value, query, lens, W1, W2, b, w):
    lens = np.asarray(lens, dtype=np.int32)
    assign, pattern = plan_assignment(lens)
    nc = _get_nc(pattern)
    in_maps = make_in_maps(value, query, lens, W1, W2, b, w, assign)
    out = None
    for attempt in range(3):
        try:
            res = run_bass_kernel_spmd(
                nc, in_maps, core_ids=list(range(NCORES))
            )
        except Exception:
            _axon_reset()
            if attempt == 2:
                raise
            continue
        out = np.empty((B, DV), np.float32)
        for i in range(NCORES):
            o = np.asarray(res.results[i]["out"], dtype=np.float32)
            for j in range(BL):
                out[assign[i][j]] = o[j, 0:DV] / o[j, DV]
        if np.isfinite(out).all():
            break
    return out



# revision 65
# speedup vs baseline: 1.1655x; 1.1655x over previous
"""Additive-attention pooling kernel for 8 TRN2 NeuronCores.

Per full input:
    u = tanh(value @ W1^T + query @ W2^T + b)          # [B, S, H]
    scores = u @ w, masked to s < lens[b], softmax over s
    out = sum_s softmax(scores)[b, s] * value[b, s, :]  # [B, DV]

Design (vs the 90.9us session baseline):

1. Length-adaptive work elision: whole 512-position units past lens[b]
   are neither loaded nor computed.  kernel() deals batches onto cores
   by active-unit count (rank-dealt slot-max pattern, provably minimal
   for whole-batch slots); one program per pattern, cached.

2. Value loads as SWDGE cast-DMAs (f32->bf16) with position layout
   s = 4p + q inside each unit, making every partition's source rows
   4KB-contiguous (4x fewer descriptors -- gpsimd descriptor generation
   was the DMA serializer).  The first chunk is split into quarter-unit
   DMAs to cut the PE head-wait.  Identities and the mask iota are
   host-packed into params so the gpsimd queue never swaps ucode
   libraries mid-stream.

3. fp8 DoubleRow u-matmuls: W1 is packed host-side as the DR lhsT
   [p, i(=k-parity), hh, m]; the f32-bitcast pair transposes feed a
   strided DVE cast-copy that de-interleaves v = 2c + i pairs into the
   plane-separated [K, 2, N] rhs DoubleRow requires.  Halves the
   u-matmul stream (rel err 1.81e-2, within the 2e-2 gate; flip
   fp8=False in build_nc for the bf16 fallback at ~3.4e-3).

4. Single-bank score accumulation (both h-halves accumulate into one
   PSUM bank per 32-row sub band -> one exp instead of exp*exp),
   partition-sliced e-transposes, inline per-half-batch pooling into
   col-group bands of one PSUM bank, and per-slot unnormalized
   (pool, denominator) outputs combined on the host (exact, since
   scores use plain exp without max subtraction).
   PSUM: tp 2 banks, up 2x[128,1024], se 1, pool 1.
"""

import numpy as np
import ml_dtypes

import concourse.bass as bass
import concourse.bacc as bacc
import concourse.tile as tile
from concourse import mybir
from concourse.bass_utils import run_bass_kernel_spmd


B, S, DV, DQ, H = 32, 4096, 256, 256, 256
NCORES = 8
BL = B // NCORES  # batches per core

ST = 32           # s-tiles of 128 per batch
TPC = 8           # s-tiles per pipeline chunk
NCHUNK = BL * (ST // TPC)  # 16 chunks per core
PW = 2508         # params: w1t 512 | w2t 512 | w 2 | b 2 | qT 8 | w1 fp8
                  # bytes 128 | ident_f32 128 | ident_bf 64 | iota_s 4x32
                  # | slot-0 units 0-1 value bf16 1024 (static DMA beats
                  # the SWDGE first-chunk latency by ~5us)
F32 = mybir.dt.float32
BF16 = mybir.dt.bfloat16
F8 = mybir.dt.float8e4
I32 = mybir.dt.int32


def build_nc(pattern=(4, 4, 4, 4), fp8=True):
    nc = bacc.Bacc("TRN2", target_bir_lowering=False)

    value_ext = nc.declare_dram_parameter("value", [BL, S, DV], F32, isOutput=False)
    lens_ext = nc.declare_dram_parameter("lens", [BL], I32, isOutput=False)
    params_ext = nc.declare_dram_parameter("params", [128, PW], F32, isOutput=False)
    out_ext = nc.declare_dram_parameter("out", [BL, DV + 1], F32,
                                        isOutput=True)

    Tanh = mybir.ActivationFunctionType.Tanh
    Exp = mybir.ActivationFunctionType.Exp
    Alu = mybir.AluOpType
    DR = mybir.MatmulPerfMode.DoubleRow

    with tile.TileContext(nc) as tc:
        with (
            tc.tile_pool(name="singles", bufs=1) as singles,
            tc.tile_pool(name="nat", bufs=BL) as nat_pool,
            tc.tile_pool(name="vt", bufs=5) as vt_pool,
            tc.tile_pool(name="ut", bufs=5) as ut_pool,
            tc.tile_pool(name="esb", bufs=3) as esb_pool,
            tc.tile_pool(name="erep", bufs=4) as erep_pool,
        ):
            # ---- first two value DMAs ahead of the iotas so the DMA
            # engines start filling chunk 0 during the iota/identity prep ----
            nat = []
            for b in range(BL):
                natb = nat_pool.tile([128, ST, DV], BF16, tag="nat",
                                     name=f"nat{b}")
                nat.append(natb)
            # position layout within each 512-unit: s = 4p + q, so each
            # partition's source rows are 4KB-contiguous (4x fewer SWDGE
            # descriptors than the s = 128t + p layout).  The first chunk
            # is split into quarter-unit pieces so its first transposes
            # start ~3us earlier (the PE head-wait is one piece, not a
            # whole 256KB unit).
            # units 0-1 of slot 0 ride the params static DMA (chunk 0
            # never waits on SWDGE)

            # ---- remaining value loads: SWDGE cast-DMAs, 256KB each,
            # in round-robin (consumption) order across slots ----
            maxch = max(pattern)
            for ch in range(maxch):
                for b in range(BL):
                    if ch >= pattern[b] or (b == 0 and ch < 2):
                        continue
                    src = value_ext[b, ch * 512:(ch + 1) * 512, :]
                    nc.gpsimd.dma_start(
                        out=nat[b][:, ch * 4:(ch + 1) * 4, :],
                        in_=src.rearrange("(p q) v -> p q v", q=4),
                    )

            # split the params load into two SEPARATE tiles so readers of
            # the transpose-critical slice (w1 fp8 | identities | iota |
            # embedded value units, cols 1036:2508, sync queue) never wait
            # on the other slice's DMA (w1t/w2t/w/b/qT, scalar queue)
            params_a = singles.tile([128, PW - 1036], F32, tag="params_a")
            nc.sync.dma_start(out=params_a, in_=params_ext[:, 1036:PW])
            params_b = singles.tile([128, 1036], F32, tag="params_b")
            nc.scalar.dma_start(out=params_b, in_=params_ext[:, 0:1036])

            w1t_f = params_b[:, 0:512].rearrange("p (c h) -> p c h", c=2)
            w2t_f = params_b[:, 512:1024].rearrange("p (c h) -> p c h", c=2)
            w_f = params_b[:, 1024:1026]
            b_sb = params_b[:, 1026:1028]
            qT = params_b[:, 1028:1036].rearrange("p (c b) -> p c b", c=2)
            w1q = params_a[:, 0:128].bitcast(F8).rearrange(
                "p (i hh m) -> p i hh m", i=2, hh=2
            )
            # identities + mask iota are host-packed: keeps the gpsimd
            # queue free for SWDGE descriptor generation (no iota library
            # swap mid-stream)
            ident_f32 = params_a[:, 128:256]
            ident_bf = params_a[:, 256:320].bitcast(BF16)
            # per-slot absolute-position iota
            iota_s = params_a[:, 320:448].rearrange(
                "p (sl t) -> p sl t", sl=BL
            )
            nat00 = params_a[:, 448:1472].bitcast(BF16).rearrange(
                "p (t v) -> p t v", t=8
            )

            def nat_tile(b, t):
                if b == 0 and t < 8:
                    return nat00[:, t, :]
                return nat[b][:, t, :]

            lens_i = singles.tile([128, BL], I32, tag="lens_i")
            nc.sync.dma_start(
                out=lens_i,
                in_=bass.AP(tensor=lens_ext, offset=0, ap=[[0, 128], [1, BL]]),
            )
            lens_f = singles.tile([128, BL], F32, tag="lens_f")
            nc.vector.tensor_copy(lens_f, lens_i)

            w1t_bf = None
            if not fp8:
                w1t_bf = singles.tile([128, 2, H], BF16, tag="w1t_bf")
                nc.vector.tensor_copy(w1t_bf, w1t_f)

            # w replicated to 32 columns for the M=32 scores matmuls
            zero32 = singles.tile([128, 32], BF16, tag="zero32")
            nc.vector.memset(zero32, 0.0)
            w_rep = singles.tile([128, 2, 32], BF16, tag="w_rep")
            for hh in range(2):
                nc.vector.tensor_scalar(
                    w_rep[:, hh, :], zero32, w_f[:, hh:hh + 1], None, Alu.add
                )

            # 1/32-filled stationary: the mask op's accum_out counts each e
            # 32x (broadcast reps), the denominator matmul divides it back
            ones_rep = singles.tile([128, 32], BF16, tag="ones_rep")
            nc.vector.memset(ones_rep, 1.0 / 32.0)

            # c[b, h] = query[b] @ W2^T + b   ->  cT [128h, hh, b] f32
            cT = singles.tile([128, 2, BL], F32, tag="cT")
            with tc.tile_pool(name="ct_ps", bufs=2, space="PSUM") as ct_pool:
                for hh in range(2):
                    ct_ps = ct_pool.tile([128, BL], F32, tag="ct")
                    for c in range(2):
                        nc.tensor.matmul(
                            ct_ps,
                            w2t_f[:, c, hh * 128:(hh + 1) * 128],
                            qT[:, c, :],
                            start=(c == 0),
                            stop=(c == 1),
                        )
                    nc.vector.tensor_scalar(
                        cT[:, hh, :], ct_ps, b_sb[:, hh:hh + 1], None, Alu.add
                    )

            psums = singles.tile([128, BL, 8], F32, tag="psums")
            nc.vector.memset(psums, 0.0)
            psum_r = singles.tile([128, BL], F32, tag="psum_r")
            psum_bf = singles.tile([128, BL], BF16, tag="psum_bf")
            out_sb = singles.tile([128, DV + 1], F32, tag="out_sb")

            # pattern[b] counts 4-tile (512-position) units; chunks pair
            # them, with a trailing half-chunk for odd unit counts
            chunk_list = []
            nchs = [(pattern[b] + 1) // 2 for b in range(BL)]
            for g in range(max(nchs) if nchs else 0):
                for b in range(BL):
                    if g >= nchs[b]:
                        continue
                    nsub = 2 if 2 * g + 2 <= pattern[b] else 1
                    chunk_list.append((b, g, nsub))
            hbs = []
            for b in range(BL):
                ks = [i for i, c in enumerate(chunk_list) if c[0] == b]
                for hbl in range(0, len(ks), 2):
                    grp = ks[hbl:hbl + 2]
                    subs = [(k, sc) for k in grp
                            for sc in range(chunk_list[k][2])]
                    hbs.append((b, hbl // 2, grp, subs))
            s_at, exp_at, et_at = {}, {}, {}
            for hb, (b, hbl, grp, subs) in enumerate(hbs):
                close = max(grp)
                s_at[close + 1] = hb
                exp_at[close + 2] = hb
                et_at[close + 3] = hb

            uts = [None] * len(chunk_list)
            vts = {}
            se_tiles = {}
            esb_tiles = {}
            erep = [None] * BL
            tmax = [4 * pattern[b] for b in range(BL)]

            with (
                tc.tile_pool(name="tp_ps", bufs=2, space="PSUM") as tp_pool,
                tc.tile_pool(name="up_ps", bufs=2, space="PSUM") as up_pool,
                tc.tile_pool(name="se_ps", bufs=1, space="PSUM") as se_pool,
                tc.tile_pool(name="po_ps", bufs=1, space="PSUM") as po_pool,
            ):
                po_ps = po_pool.tile([128, 512], F32, tag="po")

                def emit_T(k):
                    b, g, nsub = chunk_list[k]
                    if fp8:
                        # f32-bitcast pair transposes (one 2-pass LDW moves
                        # both v-parities); the PSUM->SBUF cast copy
                        # de-interleaves pairs into the plane-separated
                        # vt[c, sub, i, pos] (v = 2c + i) [K, 2, N] rhs
                        # form DoubleRow requires
                        vt = vt_pool.tile([128, TPC // 4, 2, 512], F8,
                                          tag="vt", name=f"vt{k}")
                        vts[k] = vt
                        for h in range(nsub):
                            tp = tp_pool.tile([128, 4, 128], F32, tag="tp",
                                              name=f"tp{k}_{h}")
                            for tl in range(4):
                                t = TPC * g + 4 * h + tl
                                nc.tensor.matmul(
                                    tp[:, tl, :],
                                    nat_tile(b, t).bitcast(F32),
                                    ident_f32,
                                    is_transpose=True,
                                    start=(tl == 0),
                                    stop=(tl == 3),
                                )
                            tpb = tp.bitcast(BF16)
                            src = bass.AP(
                                tensor=tpb.tensor, offset=tpb.offset,
                                ap=[tpb.ap[0], [1, 2], [256, 4], [2, 128]],
                            )
                            nc.vector.tensor_copy(vt[:, h, :, :], src)
                        return
                    vt = vt_pool.tile([128, TPC, 256], BF16, tag="vt",
                                      name=f"vt{k}")
                    vts[k] = vt
                    for h in range(nsub):
                        tp = tp_pool.tile([128, 4, 128], F32, tag="tp",
                                          name=f"tp{k}_{h}")
                        for tl in range(4):
                            t = TPC * g + 4 * h + tl
                            nc.tensor.matmul(
                                tp[:, tl, :],
                                nat_tile(b, t).bitcast(F32),
                                ident_f32,
                                is_transpose=True,
                                start=(tl == 0),
                                stop=(tl == 3),
                            )
                        nc.vector.tensor_copy(
                            vt[:, 4 * h:4 * h + 4, :], tp.bitcast(BF16)
                        )

                def emit_U(k):
                    b, g, nsub = chunk_list[k]
                    vt_base = vts[k][:, :, :]
                    ut = ut_pool.tile([128, 2, 1024], BF16, tag="ut",
                                      name=f"ut{k}")
                    uts[k] = ut
                    for hh in range(2):
                        up = up_pool.tile([128, 1024], F32, tag="up",
                                          name=f"up{k}_{hh}")
                        for sc in range(nsub):
                            if fp8:
                                nc.tensor.matmul(
                                    up[:, sc * 512:(sc + 1) * 512],
                                    w1q[:, :, hh, :],
                                    vts[k][:, sc, :, :],
                                    start=True,
                                    stop=True,
                                    perf_mode=DR,
                                )
                            else:
                                for i in range(2):
                                    rhs = bass.AP(
                                        tensor=vt_base.tensor,
                                        offset=vt_base.offset + sc * 1024 + i,
                                        ap=[vt_base.ap[0], [2, 512]],
                                    )
                                    nc.tensor.matmul(
                                        up[:, sc * 512:(sc + 1) * 512],
                                        w1t_bf[:, i, hh * 128:(hh + 1) * 128],
                                        rhs,
                                        start=(i == 0),
                                        stop=(i == 1),
                                    )
                        nc.scalar.activation(
                            ut[:, hh, 0:512 * nsub], up[:, 0:512 * nsub],
                            Tanh, bias=cT[:, hh, b:b + 1], scale=1.0,
                        )

                def emit_S(hb):
                    # both h-halves accumulate into one PSUM bank per band;
                    # hh-outer order puts the 4 distinct col-groups
                    # back-to-back so their streams overlap in the array
                    b, hbl, grp, subs = hbs[hb]
                    se_t = se_pool.tile([128, 512], F32, tag="se",
                                        name=f"se{hb}")
                    se_tiles[hb] = se_t
                    for hh in range(2):
                        for r, (j, sc) in enumerate(subs):
                            nc.tensor.matmul(
                                se_t[32 * r:32 * r + 32, :],
                                w_rep[:, hh, :],
                                uts[j][:, hh, sc * 512:(sc + 1) * 512],
                                start=(hh == 0),
                                stop=(hh == 1),
                                tile_position=(0, 32 * r),
                            )

                def emit_EXP(hb):
                    L = len(hbs[hb][3])
                    se_t = se_tiles[hb]
                    e_sb = esb_pool.tile([128, 512], BF16, tag="esb",
                                         name=f"esb{hb}")
                    esb_tiles[hb] = e_sb
                    nc.scalar.activation(
                        e_sb[0:32 * L, :], se_t[0:32 * L, :], Exp
                    )

                def emit_ET(hb):
                    b, hbl, grp, subs = hbs[hb]
                    L = len(subs)
                    e_sb = esb_tiles[hb]
                    et = tp_pool.tile([128, 1024], BF16, tag="tp",
                                      name=f"et{hb}")
                    for tl in range(4):
                        nc.tensor.matmul(
                            et[:, tl * 128:tl * 128 + 32 * L],
                            e_sb[0:32 * L, 128 * tl:128 * (tl + 1)],
                            ident_bf[0:32 * L, 0:32 * L],
                            is_transpose=True,
                            start=(tl == 0),
                            stop=(tl == 3),
                        )
                    # e_rep[b][:, t, :] = mask * e broadcast to 32 reps, for
                    # t = 16*(hb%2) + 8*r1 + 4*r0 + tl at et[:, 256*tl + 32*(2*r1+r0)]
                    if hbl == 0:
                        erep[b] = erep_pool.tile([128, ST, 32], BF16,
                                                 tag="erep", name=f"erep{b}")
                    et_all = et[:, :]
                    for r in range(len(subs)):
                        tbase = 16 * hbl + 4 * r
                        in1 = bass.AP(
                            tensor=et_all.tensor, offset=et_all.offset + 32 * r,
                            ap=[et_all.ap[0], [128, 4], [0, 32]],
                        )
                        iosl = iota_s[:, b, tbase:tbase + 4]
                        io_b = bass.AP(
                            tensor=iosl.tensor, offset=iosl.offset,
                            ap=[iosl.ap[0], [1, 4], [0, 32]],
                        )
                        osl = erep[b][:, tbase:tbase + 4, :]
                        slot = hbl * 4 + r
                        nc.vector.scalar_tensor_tensor(
                            osl,
                            io_b,
                            lens_f[:, b:b + 1],
                            in1,
                            Alu.is_lt,
                            Alu.mult,
                            accum_out=psums[:, b, slot:slot + 1],
                        )

                pool_pending = {b: [] for b in range(BL)}

                def emit_pool_mm(b, t):
                    nc.tensor.matmul(
                        po_ps[32 * b:32 * b + 32, 0:DV],
                        erep[b][:, t, :],
                        nat_tile(b, t),
                        start=(t == 0),
                        stop=(t == tmax[b] - 1),
                        tile_position=(0, 32 * b),
                    )
                    if t == tmax[b] - 1:
                        emit_OUT(b)

                def pool_step(force=False):
                    # drain pending pool tiles round-robin across batches:
                    # adjacent matmuls then target different col-groups and
                    # their N=256 streams overlap in the array (~2x).
                    # Tiles of a lone batch are held until a second batch
                    # has pending work (or the final forced drain).
                    while True:
                        active = [b for b in range(BL) if pool_pending[b]]
                        if not active or (len(active) < 2 and not force):
                            break
                        for b in active:
                            emit_pool_mm(b, pool_pending[b].pop(0))

                def emit_OUT(b):
                    # per-slot unnormalized (pool, denominator) copy-out
                    # as soon as this slot's pool accumulation closes;
                    # the host divides after summing across cores/slots
                    rows = slice(32 * b, 32 * b + 32)
                    nc.vector.tensor_reduce(
                        psum_r[:, b:b + 1], psums[:, b, :], op=Alu.add,
                        axis=mybir.AxisListType.X,
                    )
                    nc.vector.tensor_copy(
                        psum_bf[:, b:b + 1], psum_r[:, b:b + 1]
                    )
                    nc.tensor.matmul(
                        po_ps[rows, DV:DV + 1],
                        ones_rep,
                        psum_bf[:, b:b + 1],
                        start=True,
                        stop=True,
                        tile_position=(0, 32 * b),
                    )
                    nc.vector.tensor_copy(
                        out_sb[rows], po_ps[rows, 0:DV + 1]
                    )

                pool_at = {}
                for hb, (b, hbl, grp, subs) in enumerate(hbs):
                    pool_at[max(grp) + 4] = hb

                NK = len(chunk_list)
                for k in range(NK + 5):
                    if k in exp_at:
                        emit_EXP(exp_at[k])
                    if k < NK:
                        emit_T(k)
                    if k in et_at:
                        emit_ET(et_at[k])
                    if k < NK:
                        emit_U(k)
                    if k in s_at:
                        emit_S(s_at[k])
                    if k in pool_at:
                        hb = pool_at[k]
                        b, hbl, grp, subs = hbs[hb]
                        pool_pending[b].extend(
                            range(16 * hbl, 16 * hbl + 4 * len(subs))
                        )
                    pool_step()
                pool_step(force=True)
                for b in range(BL):
                    if pattern[b] == 0:
                        # phantom slot: emit zeros so the host-side sum
                        # is unaffected
                        nc.vector.memset(
                            out_sb[32 * b:32 * b + 32, :], 0.0
                        )

                ob_rows = out_sb.rearrange("(a b) s -> a b s", b=32)[:, 0, :]
                nc.sync.dma_start(out=out_ext[:, :], in_=ob_rows)

    nc.compile()
    return nc


_NC_CACHE = {}


def _get_nc(pattern):
    if pattern not in _NC_CACHE:
        _NC_CACHE[pattern] = build_nc(pattern)
    return _NC_CACHE[pattern]


def plan_assignment(lens):
    """Deal batches (sorted by active chunks, desc) round-robin into cores.

    Returns (assign[core][slot] = batch index, pattern) where pattern[j] is
    the max chunk count over cores at slot j -- the shared compiled shape
    (provably minimal for whole-batch slots).
    """
    g = np.clip(np.ceil(np.asarray(lens) / 512).astype(int), 1, 8)
    order = np.argsort(-g, kind="stable")
    assign = [[None] * BL for _ in range(NCORES)]
    for rank, b in enumerate(order):
        assign[rank % NCORES][rank // NCORES] = int(b)
    pattern = tuple(
        int(max(g[assign[i][j]] for i in range(NCORES))) for j in range(BL)
    )
    return assign, pattern


def make_in_maps(value, query, lens, W1, W2, b, w, assign):
    value = np.ascontiguousarray(np.asarray(value, dtype=np.float32))
    query = np.asarray(query, dtype=np.float32)
    lens = np.ascontiguousarray(np.asarray(lens, dtype=np.int32))
    w1t = np.asarray(W1, dtype=np.float32).T
    f8 = ml_dtypes.float8_e4m3
    w1_q = np.asarray(W1, dtype=np.float32).astype(f8)   # [h, v]
    # [p, i, hh, m] = W1_q[128*hh + m, 2p + i]
    wq = w1_q.reshape(2, 128, 128, 2)                     # [hh, m, p, i]
    wq = np.ascontiguousarray(wq.transpose(2, 3, 0, 1))   # [p, i, hh, m]
    w1_bytes = wq.reshape(128, 512).view(np.float32)      # [128, 128]
    w2t = np.asarray(W2, dtype=np.float32).T
    bvec = np.asarray(b, dtype=np.float32).reshape(H)
    wvec = np.asarray(w, dtype=np.float32).reshape(H)

    def pack(core):
        sel = assign[core]
        P = np.zeros((128, PW), np.float32)
        # w1t by v-parity: col (i*256 + hh*128 + m) = W1T[2p+i, 128hh+m]
        w1p = w1t.reshape(128, 2, 2, 128)          # [p2, i, hh, m] with v=2*p2+i
        P[:, 0:512] = w1p.transpose(0, 1, 2, 3).reshape(128, 512)
        P[:, 512:1024] = w2t.reshape(2, 128, H).transpose(1, 0, 2).reshape(128, 512)
        P[:, 1024:1026] = wvec.reshape(2, 128).T
        P[:, 1026:1028] = bvec.reshape(2, 128).T
        P[:, 1028:1036] = (
            query[sel].T.reshape(2, 128, BL).transpose(1, 0, 2)
            .reshape(128, 2 * BL)
        )
        P[:, 1036:1164] = w1_bytes
        P[:, 1164:1292] = np.eye(128, dtype=np.float32)
        P[:, 1292:1356] = (
            np.eye(128, dtype=ml_dtypes.bfloat16).reshape(128, 64, 2)
            .view(np.float32).reshape(128, 64)
        )
        # mask iota per slot: tile t = (u, q) holds position 512u + 4p + q
        p_i = np.arange(128)[:, None]
        u_i = (np.arange(32) // 4)[None, :]
        q_i = (np.arange(32) % 4)[None, :]
        io = (512 * u_i + 4 * p_i + q_i).astype(np.float32)
        P[:, 1356:1484] = np.tile(io, (1, BL))
        # slot-0 units 0-1 value as bf16 in the s = 4p + q layout
        v00 = value[sel[0], 0:1024, :].astype(ml_dtypes.bfloat16)
        P[:, 1484:2508] = (
            v00.reshape(2, 128, 4, 256).transpose(1, 0, 2, 3)
            .reshape(128, 2048).view(np.float32)
        )
        return np.ascontiguousarray(P)

    in_maps = []
    for i in range(NCORES):
        sel = assign[i]
        in_maps.append({
            "value": np.ascontiguousarray(value[sel]),
            "lens": np.ascontiguousarray(lens[sel]),
            "params": pack(i),
        })
    return in_maps


def _axon_reset():
    # clear a wedged exec unit left over from a previous crashed run
    try:
        import ctypes
        import jax
        jax.devices()
        lib = ctypes.CDLL("/opt/axon/libaxon_pjrt.so")
        lib.axon_reset.restype = ctypes.c_int64
        lib.axon_reset()
    except Exception:
        pass


def kernel(value, query, lens, W1, W2, b, w):
    lens = np.asarray(lens, dtype=np.int32)
    assign, pattern = plan_assignment(lens)
    nc = _get_nc(pattern)
    in_maps = make_in_maps(value, query, lens, W1, W2, b, w, assign)
    out = None
    for attempt in range(3):
        try:
            res = run_bass_kernel_spmd(
                nc, in_maps, core_ids=list(range(NCORES))
            )
        except Exception:
            _axon_reset()
            if attempt == 2:
                raise
            continue
        out = np.empty((B, DV), np.float32)
        for i in range(NCORES):
            o = np.asarray(res.results[i]["out"], dtype=np.float32)
            for j in range(BL):
                out[assign[i][j]] = o[j, 0:DV] / o[j, DV]
        if np.isfinite(out).all():
            break
    return out

